# revision 4
# baseline (speedup 1.0000x reference)
"""Trainium2 Bass kernel v9: dense transformer block (pre-LN, causal MHA+FFN).

Math per batch elem b (T=64, D=384): h=LN(x); per-head QKV; causal softmax;
attn@wo; h+=attn; h2=LN(h); out = h2 + relu(h2@w1)@w2.

Sharding: pure data parallel over batch (2048 -> 256/core, 8 cores SPMD).

v9 (from the v8 f32r baseline, HW 2.31ms -> ~1.6ms steady):
  - QKV/V/Wo/FFN2 GEMMs in fp8e4m3 with DoubleRow perf mode: contraction
    pairs packed 2/PE-cell (K=256/pass).  D=384 contractions run as one DR
    pass (256) + one plain-fp8 pass (128); DP=512 and F=1536 contractions
    are clean multiples of 256.  Weights pre-scaled x64 into fp8 range;
    descales folded into existing PSUM->SBUF copies / act scales.  FFN1
    stays bf16 -- it was the largest fp8 error contributor (rel err
    1.55e-2 -> 1.35e-2 vs the 2e-2 gate) and bf16 costs only ~2us/tile.
  - attention interior (scores, probs transposes, attnV) in bf16
    (1 cyc/row vs fp32's 4) with the same 64x64 PE-quadrant layout.
  - causal mask folded into the scores PSUM by an identity-matmul writing
    -1e30 biases (start of each accumulation group) -- no DVE mask multiply.
  - residuals folded into the Wo / FFN2 PSUM groups as scaled-identity
    matmuls (id*4096 @ h, id*64 @ h2) -- no DVE residual adds.
  - all token-major activation tiles bf16: DVE TensorScalar/Copy run in
    2x/4x perf modes; LN rstd via Quake+Newton on DVE (no Act table swaps;
    Act runs only Exp/Copy/Relu).
  - rotated software pipeline (emission order per iteration): pass2+Wo(it),
    LN1(it+1), transposes+QKV+V(it+1), LN2(it), h2T(it), scores+softmax
    (it+1), FFN(it).  Next-tile PE work fills the LN/softmax windows; buffer
    depths (p_h=10, p_qk/p_v=2, p_sm=8) keep the in-order engine streams
    from head-of-line blocking across tiles.
"""

import sys

sys.path.insert(0, "/opt/trn_rl_repo")

import numpy as np

import concourse.bass as bass
import concourse.tile as tile
from concourse import mybir

# ---- problem constants (hardcoded per contract) ----
B_TOTAL = 2048
T = 64
D = 384
H = 8
E = 48
EP = 64
F = 4 * D
N_CORES = 8
B_CORE = B_TOTAL // N_CORES
LN_EPS = 1e-5
INV_SQRT_E = float(E) ** -0.5

NB = 8
NT = NB * T  # 512
KC = D // 128  # 3
FC = F // 128  # 12
TC = NT // 128  # 4
DP = H * EP  # 512
WS = 64.0  # fp8 weight pre-scale
MBIAS = -1e30  # causal mask bias (pre exp-scale)

F32 = mybir.dt.float32
I32 = mybir.dt.int32
BF16 = mybir.dt.bfloat16
FP8 = mybir.dt.float8e4

QUAKE_MAGIC = 0x5F3759DF
DRM = mybir.MatmulPerfMode.DoubleRow


def build_body(tc, aps, b_core):
    from contextlib import ExitStack

    ctx = ExitStack()
    nc = tc.nc
    n_tiles = b_core * T // NT

    x_dr = aps["x"].rearrange("b t d -> (b t) d")
    out_dr = aps["out"].rearrange("b t d -> (b t) d")

    AF = mybir.ActivationFunctionType
    OP = mybir.AluOpType
    flags = aps["flags"]

    singles = ctx.enter_context(tc.tile_pool(name="singles", bufs=1))

    def load_const(name, shape, src_ap, dt):
        t_ = singles.tile(list(shape), dt, name=f"sb_{name}")
        nc.sync.dma_start(out=t_, in_=src_ap)
        return t_

    ident1 = load_const("ident1", [128, 128], aps["ident1"], BF16)
    id4096 = load_const("id4096", [128, 128], aps["id4096"], BF16)
    id64 = load_const("id64", [128, 128], aps["id64"], BF16)
    maskb = load_const("maskb", [128, 4 * T], aps["maskb"], BF16)
    quake = load_const("quake", [128, 8], aps["quake"], I32)

    wqk_dr = {
        (qi, ch): load_const(f"wqkd{qi}{ch}", [128, 256], aps["wqk_dr"][qi, ch], FP8)
        for qi in range(2)
        for ch in range(4)
    }
    wqk_p = {
        (qi, ch): load_const(f"wqkp{qi}{ch}", [128, 128], aps["wqk_p"][qi, ch], FP8)
        for qi in range(2)
        for ch in range(4)
    }
    wv_dr = load_const("wvd", [128, 2 * DP], aps["wv_dr"], FP8)
    wv_p = load_const("wvp", [128, DP], aps["wv_p"], FP8)
    w1 = {
        (k, f): load_const(f"w1_{k}_{f}", [128, 128], aps["w1"][k, f], BF16)
        for k in range(KC)
        for f in range(FC)
    }
    w2_dr = {
        j: load_const(f"w2d{j}", [128, 2 * D], aps["w2_dr"][j], FP8) for j in range(6)
    }
    wo_dr = {
        j: load_const(f"wod{j}", [128, 2 * D], aps["wo_dr"][j], FP8) for j in range(2)
    }

    bqk = load_const("bqk", [128, 8], aps["bqk"], F32) if flags["bqk"] else None
    bv_b = load_const("bv_b", [128, DP], aps["bv_b"], F32) if flags["bv"] else None
    b1c = load_const("b1c", [128, FC], aps["b1c"], F32) if flags["b1"] else None
    g1_b = load_const("g1_b", [128, D], aps["g1_b"], F32) if flags["g1be1"] else None
    be1_b = load_const("be1_b", [128, D], aps["be1_b"], F32) if flags["g1be1"] else None
    g2_b = load_const("g2_b", [128, D], aps["g2_b"], F32) if flags["g2be2"] else None
    be2_b = load_const("be2_b", [128, D], aps["be2_b"], F32) if flags["g2be2"] else None
    bo_b = load_const("bo_b", [128, D], aps["bo_b"], F32) if flags["bo"] else None
    b2_b = load_const("b2_b", [128, D], aps["b2_b"], F32) if flags["b2"] else None

    pool = lambda nm, n, **kw: ctx.enter_context(tc.tile_pool(name=nm, bufs=n, **kw))
    ps = pool("ps", 4, space="PSUM")
    ps_at = pool("ps_at", 1, space="PSUM")
    p_x = pool("p_x", 4)
    p_h = pool("p_h", 10)  # 4/tile; stage_a(it+1) overlaps Wo(it) readers
    p_hT = pool("p_hT", 1)  # tags hT01/hT2/h2T01/h2T2
    p_qk = pool("p_qk", 2)  # 8 tags; next tile's QKV overlaps scores(it)
    p_v = pool("p_v", 2)  # 8 tags; next tile's V overlaps attnV(it)
    p_sm = pool("p_sm", 8)
    p_pt = pool("p_pt", 3)
    p_at = pool("p_at", 1)  # 2 tags
    p_hr = pool("p_hr", 5)
    p_h2 = pool("p_h2", 6)
    p_rel = pool("p_rel", 1)  # 6 tags
    p_out = pool("p_out", 2)
    p_st = pool("p_st", 4)

    def rsqrt_dve(ve_view, n, tag):
        """rstd[128, n] = 1/sqrt(ve + eps): Quake bitcast + 2 Newton on DVE."""
        ve = p_st.tile([128, n], F32, tag=f"q0{tag}", name=f"q0{tag}")
        nc.vector.tensor_scalar_add(out=ve, in0=ve_view, scalar1=LN_EPS)
        t1 = p_st.tile([128, n], I32, tag=f"q1{tag}", name=f"q1{tag}")
        nc.vector.tensor_tensor(
            out=t1, in0=ve.bitcast(I32), in1=quake[:, 4 : 4 + n], op=OP.logical_shift_right
        )
        y0 = p_st.tile([128, n], F32, tag=f"q2{tag}", name=f"q2{tag}")
        nc.vector.tensor_tensor(
            out=y0.bitcast(I32), in0=quake[:, 0:n], in1=t1, op=OP.subtract
        )
        y = y0
        for ni in range(2):
            a = p_st.tile([128, n], F32, tag=f"q3{tag}{ni}", name=f"q3{tag}{ni}")
            nc.vector.tensor_mul(out=a, in0=y, in1=y)
            b = p_st.tile([128, n], F32, tag=f"q4{tag}{ni}", name=f"q4{tag}{ni}")
            nc.vector.scalar_tensor_tensor(
                out=b, in0=a, scalar=-0.5, in1=ve, op0=OP.mult, op1=OP.mult
            )
            c = p_st.tile([128, n], F32, tag=f"q5{tag}{ni}", name=f"q5{tag}{ni}")
            nc.vector.tensor_scalar_add(out=c, in0=b, scalar1=1.5)
            yn = p_st.tile([128, n], F32, tag=f"q6{tag}{ni}", name=f"q6{tag}{ni}")
            nc.vector.tensor_mul(out=yn, in0=y, in1=c)
            y = yn
        return y

    def layernorm_group(x_ts, g_b, be_b, gb_nontriv, pool_, tag):
        """LN over TC token chunks -> bf16 outputs; batched rstd."""
        mv = p_st.tile([128, 2 * TC], F32, tag=f"mv{tag}", name=f"mv_{tag}")
        for c in range(TC):
            st = p_st.tile([128, 6], F32, tag="st", name=f"st_{tag}")
            nc.vector.bn_stats(out=st, in_=x_ts[c])
            nc.vector.bn_aggr(out=mv[:, 2 * c : 2 * c + 2], in_=st)
        mv3 = mv.rearrange("p (c two) -> p c two", two=2)
        rstd = rsqrt_dve(mv3[:, :, 1], TC, tag)
        nmr = p_st.tile([128, TC], F32, tag=f"nm{tag}", name=f"nm_{tag}")
        nc.vector.scalar_tensor_tensor(
            out=nmr, in0=mv3[:, :, 0], scalar=-1.0, in1=rstd, op0=OP.mult, op1=OP.mult
        )
        h_ts = []
        for c in range(TC):
            h_t = pool_.tile([128, D], BF16, tag=tag, name=f"h_{tag}")
            nc.vector.tensor_scalar(
                out=h_t,
                in0=x_ts[c],
                scalar1=rstd[:, c : c + 1],
                scalar2=nmr[:, c : c + 1],
                op0=OP.mult,
                op1=OP.add,
            )
            if gb_nontriv:
                nc.vector.tensor_mul(out=h_t, in0=h_t, in1=g_b)
                nc.vector.tensor_add(out=h_t, in0=h_t, in1=be_b)
            h_ts.append(h_t)
        return h_ts

    def transpose_feat(h_ts, tag, dt=FP8):
        """token-major bf16 [128, D] x TC -> feature-major: pair tile
        [128, 2*NT] (chunks 0,1) + single tile [128, NT] (chunk 2).
        Each PSUM->SBUF copy is split Act/DVE so the copy pole is short."""
        t01 = p_hT.tile([128, 2 * NT], dt, tag=f"{tag}01", name=f"{tag}01")
        t2 = p_hT.tile([128, NT], dt, tag=f"{tag}2", name=f"{tag}2")
        for k in range(KC):
            pt_ = ps.tile([128, NT], BF16, tag="ps", name=f"tp_{tag}")
            for c in range(TC):
                nc.tensor.transpose(
                    out=pt_[:, c * 128 : (c + 1) * 128],
                    in_=h_ts[c][:, k * 128 : (k + 1) * 128],
                    identity=ident1,
                )
            dst = t2 if k == 2 else t01[:, k * NT : (k + 1) * NT]
            if dt == FP8:
                nc.scalar.copy(out=dst, in_=pt_)
            else:
                nc.vector.tensor_copy(out=dst, in_=pt_)
        return t01.rearrange("p (two n) -> p two n", two=2), t2

    x_tiles = {}

    def load_x(i):
        if i >= n_tiles:
            return
        t = p_x.tile([128, TC * D], BF16, tag="x", name="x")
        nc.sync.dma_start(
            out=t.rearrange("p (c d) -> p c d", c=TC),
            in_=x_dr[i * NT : (i + 1) * NT, :].rearrange("(c p) d -> p c d", p=128),
        )
        x_tiles[i] = t

    wv_dr3 = wv_dr.rearrange("p (two n) -> p two n", two=2)

    def stage_a_ln(it):
        """LN1 for tile `it` (DVE only)."""
        x_big = x_tiles.pop(it)
        x_ts = [x_big[:, c * D : (c + 1) * D] for c in range(TC)]
        return layernorm_group(x_ts, g1_b, be1_b, flags["g1be1"], p_h, "h")

    def stage_a_qkv(h_ts):
        """transposes + QKV + V from LN1 output."""
        hT01, hT2 = transpose_feat(h_ts, "hT")

        # Q,K: feature-major, 64-padded heads; chunk ch = heads (2ch, 2ch+1)
        qk_sb = []
        for qi in range(2):
            row = []
            for ch in range(4):
                pm = ps.tile([128, NT], F32, tag="ps", name="qk_ps")
                nc.tensor.matmul(
                    out=pm,
                    lhsT=wqk_dr[(qi, ch)].rearrange("p (two m) -> p two m", two=2),
                    rhs=hT01,
                    start=True,
                    stop=False,
                    perf_mode=DRM,
                )
                nc.tensor.matmul(
                    out=pm, lhsT=wqk_p[(qi, ch)], rhs=hT2, start=False, stop=True
                )
                sb = p_qk.tile([128, NT], BF16, tag=f"qk{qi}{ch}", name=f"qk{qi}{ch}")
                if flags["bqk"]:
                    nc.scalar.activation(
                        out=sb,
                        in_=pm,
                        func=AF.Identity,
                        bias=bqk[:, qi * 4 + ch : qi * 4 + ch + 1],
                        scale=1.0,
                    )
                else:
                    nc.scalar.copy(out=sb, in_=pm)
                row.append(sb)
            qk_sb.append(row)

        # V: token-major [128 tok, DP]; values carry xWS
        v_sb, vs_sb = [], []
        for c in range(TC):
            pm = ps.tile([128, DP], F32, tag="ps", name="v_ps")
            nc.tensor.matmul(
                out=pm,
                lhsT=hT01[:, :, c * 128 : (c + 1) * 128],
                rhs=wv_dr3,
                start=True,
                stop=False,
                perf_mode=DRM,
            )
            nc.tensor.matmul(
                out=pm,
                lhsT=hT2[:, c * 128 : (c + 1) * 128],
                rhs=wv_p,
                start=False,
                stop=True,
            )
            sb = p_v.tile([128, DP], BF16, tag=f"v{c}", name=f"v{c}")
            nc.vector.tensor_copy(out=sb, in_=pm)
            if flags["bv"]:
                nc.vector.tensor_add(out=sb, in0=sb, in1=bv_b)
            sw = p_v.tile([128, DP], BF16, tag=f"vs{c}", name=f"vs{c}")
            # half-swapped copy; bf16 SBUF->SBUF runs 4x on DVE
            nc.vector.tensor_copy(out=sw[0:64, :], in_=sb[64:128, :])
            nc.vector.tensor_copy(out=sw[64:128, :], in_=sb[0:64, :])
            v_sb.append(sb)
            vs_sb.append(sw)
        return h_ts, qk_sb, v_sb, vs_sb

    load_x(0)
    load_x(1)
    load_x(2)

    exp_scale = INV_SQRT_E / (WS * WS)

    def pass1(it, sa_):
        """scores + softmax for tile it (probs stay x1, v carries xWS)."""
        _, qk_sb, _, _ = sa_
        exs = []
        for p in range(NB // 2):
            sc_par = [
                ps.tile([128, 4 * T], F32, tag="ps", name=f"sc_ps{par}")
                for par in range(2)
            ]
            for par in range(2):
                # causal mask bias written by PE: I.T @ maskb
                nc.tensor.matmul(
                    out=sc_par[par], lhsT=ident1, rhs=maskb, start=True, stop=False
                )
            for half in range(2):
                bb = 2 * p + half
                for h in range(H):
                    ch, off = h // 2, EP * (h % 2)
                    nc.tensor.matmul(
                        out=sc_par[h % 2][
                            64 * half : 64 * half + 64, (h // 2) * T : (h // 2 + 1) * T
                        ],
                        lhsT=qk_sb[0][ch][off : off + E, bb * T : (bb + 1) * T],
                        rhs=qk_sb[1][ch][off : off + E, bb * T : (bb + 1) * T],
                        start=False,
                        stop=(half == 1 and h >= H - 2),
                    )
            ex = p_sm.tile([128, 8 * T], BF16, tag="ex", name="ex")
            for par in range(2):
                nc.scalar.activation(
                    out=ex[:, par * 4 * T : (par + 1) * 4 * T],
                    in_=sc_par[par],
                    func=AF.Exp,
                    bias=0.0,
                    scale=exp_scale,
                )
            rs = p_st.tile([128, H], F32, tag="rsum", name="rsum")
            nc.vector.reduce_sum(
                out=rs,
                in_=ex.rearrange("p (h s) -> p h s", h=H),
                axis=mybir.AxisListType.X,
            )
            rr = p_st.tile([128, H], F32, tag="rrec", name="rrec")
            nc.vector.reciprocal(out=rr, in_=rs)
            for h in range(H):
                nc.vector.tensor_scalar_mul(
                    out=ex[:, h * T : (h + 1) * T],
                    in0=ex[:, h * T : (h + 1) * T],
                    scalar1=rr[:, h : h + 1],
                )
            exs.append(ex)
        return exs

    def pass2_wo(it, sa_, exs):
        """probs transposes + attnV, with per-token-chunk at copies and Wo
        groups interleaved so the PE stream never waits on a full drain."""
        h_ts, _, v_sb, vs_sb = sa_
        at_ps = {
            (g, cg): ps_at.tile([128, NT], F32, tag=f"at{g}{cg}", name=f"at{g}{cg}")
            for g in range(2)
            for cg in range(2)
        }
        at01 = p_at.tile([128, 2 * NT], FP8, tag="at01", name="at01")
        at23 = p_at.tile([128, 2 * NT], FP8, tag="at23", name="at23")
        at01_3 = at01.rearrange("p (two n) -> p two n", two=2)
        at23_3 = at23.rearrange("p (two n) -> p two n", two=2)
        hr_ts = []

        def wo_group(c):
            pm = ps.tile([128, D], F32, tag="ps", name="wo_ps")
            nc.tensor.matmul(
                out=pm,
                lhsT=at01_3[:, :, c * 128 : (c + 1) * 128],
                rhs=wo_dr[0].rearrange("p (two n) -> p two n", two=2),
                start=True,
                stop=False,
                perf_mode=DRM,
            )
            nc.tensor.matmul(
                out=pm,
                lhsT=at23_3[:, :, c * 128 : (c + 1) * 128],
                rhs=wo_dr[1].rearrange("p (two n) -> p two n", two=2),
                start=False,
                stop=False,
                perf_mode=DRM,
            )
            nc.tensor.matmul(out=pm, lhsT=id4096, rhs=h_ts[c], start=False, stop=True)
            hr = p_hr.tile([128, D], BF16, tag="hr", name="hr")
            nc.scalar.activation(
                out=hr, in_=pm, func=AF.Copy, bias=0.0, scale=1.0 / 4096.0
            )
            if flags["bo"]:
                nc.vector.tensor_add(out=hr, in0=hr, in1=bo_b)
            hr_ts.append(hr)

        for p in range(NB // 2):
            ex = exs[p]
            ptp = ps.tile([128, 8 * T], BF16, tag="ps", name="pt_ps")
            for j2 in range(4):
                nc.tensor.transpose(
                    out=ptp[:, j2 * 128 : (j2 + 1) * 128],
                    in_=ex[:, j2 * 128 : (j2 + 1) * 128],
                    identity=ident1,
                )
            ptsb = p_pt.tile([128, 8 * T], BF16, tag="pt", name="pt")
            if p % 2 == 0:
                nc.scalar.copy(out=ptsb, in_=ptp)
            else:
                nc.vector.tensor_copy(out=ptsb, in_=ptp)
            for half in range(2):
                bb = 2 * p + half
                c, hb = bb // 2, 64 * (bb % 2)
                for h in range(H):
                    ch = h // 2
                    j = (h % 2) * 4 + ch
                    pbase = 64 * (j % 2)
                    vt = v_sb[c] if (j % 2) == (bb % 2) else vs_sb[c]
                    nc.tensor.matmul(
                        out=at_ps[(ch % 2, ch // 2)][
                            EP * (h % 2) : EP * (h % 2) + EP,
                            bb * T : (bb + 1) * T,
                        ],
                        lhsT=vt[pbase : pbase + 64, h * EP : (h + 1) * EP],
                        rhs=ptsb[
                            pbase : pbase + 64,
                            (j // 2) * 128 + hb : (j // 2) * 128 + hb + 64,
                        ],
                        start=True,
                        stop=True,
                    )
        nc.scalar.copy(out=at01[:, 0:NT], in_=at_ps[(0, 0)])
        nc.vector.tensor_copy(out=at01[:, NT : 2 * NT], in_=at_ps[(1, 0)])
        nc.scalar.copy(out=at23[:, 0:NT], in_=at_ps[(0, 1)])
        nc.vector.tensor_copy(out=at23[:, NT : 2 * NT], in_=at_ps[(1, 1)])
        for c in range(TC):
            wo_group(c)
        return hr_ts

    def ffn_store(it, h2_ts, h2T):
        """FFN1 (bf16) + relu -> fp8 pairs; FFN2 (DR) + residual; store."""
        row0 = it * NT
        rel3 = []
        for j in range(6):
            rp = p_rel.tile([128, 2 * NT], FP8, tag=f"rel{j}", name=f"rel{j}")
            rel3.append(rp)
        for f in range(FC):
            pm = ps.tile([128, NT], F32, tag="ps", name="f1_ps")
            for k in range(KC):
                nc.tensor.matmul(
                    out=pm,
                    lhsT=w1[(k, f)],
                    rhs=h2T[k],
                    start=(k == 0),
                    stop=(k == KC - 1),
                )
            dst = rel3[f // 2][:, (f % 2) * NT : (f % 2 + 1) * NT]
            if flags["b1"]:
                nc.scalar.activation(
                    out=dst, in_=pm, func=AF.Relu, bias=b1c[:, f : f + 1], scale=1.0
                )
            elif f % 2 == 0:
                nc.scalar.activation(out=dst, in_=pm, func=AF.Relu, bias=0.0, scale=1.0)
            else:
                nc.vector.tensor_scalar_max(out=dst, in0=pm, scalar1=0.0)

        # ---- FFN2 + residual (id*64 @ h2) + wide store ----
        o_big = p_out.tile([128, TC * D], F32, tag="o", name="o")
        for c in range(TC):
            pm = ps.tile([128, D], F32, tag="ps", name="f2_ps")
            for j in range(6):
                nc.tensor.matmul(
                    out=pm,
                    lhsT=rel3[j].rearrange("p (two n) -> p two n", two=2)[
                        :, :, c * 128 : (c + 1) * 128
                    ],
                    rhs=w2_dr[j].rearrange("p (two n) -> p two n", two=2),
                    start=(j == 0),
                    stop=False,
                    perf_mode=DRM,
                )
            nc.tensor.matmul(out=pm, lhsT=id64, rhs=h2_ts[c], start=False, stop=True)
            o_t = o_big[:, c * D : (c + 1) * D]
            nc.scalar.activation(out=o_t, in_=pm, func=AF.Copy, bias=0.0, scale=1.0 / WS)
            if flags["b2"]:
                nc.vector.tensor_add(out=o_t, in0=o_t, in1=b2_b)
        nc.sync.dma_start(
            out=out_dr[row0 : row0 + NT, :].rearrange("(c p) d -> p c d", p=128),
            in_=o_big.rearrange("p (c d) -> p c d", c=TC),
        )

    # ---- rotated pipeline.  Emission order per iteration:
    #   pass2_wo(it): probsT/attnV with per-chunk at drains + Wo interleaved
    #   LN1(it+1) on DVE (runs under pass2/Wo PE work)
    #   stage_a_qkv(it+1): transposes+QKV+V PE stream
    #   LN2(it) on DVE (runs under QKV/scores PE work)
    #   pass1(it+1): scores fill the LN2 window; softmax overlaps FFN(it)
    #   h2T transposes + FFN(it)
    sa = {0: stage_a_qkv(stage_a_ln(0))}
    exs = {0: pass1(0, sa[0])}
    for it in range(n_tiles):
        load_x(it + 3)
        hr_ts = pass2_wo(it, sa[it], exs.pop(it))
        h_next = stage_a_ln(it + 1) if it + 1 < n_tiles else None
        if h_next is not None:
            sa[it + 1] = stage_a_qkv(h_next)
        h2_ts = layernorm_group(hr_ts, g2_b, be2_b, flags["g2be2"], p_h2, "h2")
        h2T01, h2T2 = transpose_feat(h2_ts, "h2T", dt=BF16)
        if it + 1 < n_tiles:
            exs[it + 1] = pass1(it + 1, sa[it + 1])
        ffn_store(it, h2_ts, [h2T01[:, 0, :], h2T01[:, 1, :], h2T2])
        sa.pop(it)

    ctx.close()


def prep_inputs(inputs, b_core):
    import ml_dtypes

    f32 = np.float32
    fp8 = ml_dtypes.float8_e4m3fn
    wq, wk, wvv = (np.asarray(inputs[k], f32) for k in ("wq", "wk", "wv"))
    bq, bk, bv = (np.asarray(inputs[k], f32) for k in ("bq", "bk", "bv"))
    wo, bo = np.asarray(inputs["wo"], f32), np.asarray(inputs["bo"], f32)
    w1, b1 = np.asarray(inputs["w1"], f32), np.asarray(inputs["b1"], f32)
    w2, b2 = np.asarray(inputs["w2"], f32), np.asarray(inputs["b2"], f32)
    g1, be1 = np.asarray(inputs["g1"], f32), np.asarray(inputs["be1"], f32)
    g2, be2 = np.asarray(inputs["g2"], f32), np.asarray(inputs["be2"], f32)

    def q8(a):
        return (a * WS).astype(fp8)

    # wqk DR pair (contraction chunks 0,1) + plain (chunk 2); head-pair cols
    wqk_dr = np.zeros((2, 4, 128, 256), f32)
    wqk_p = np.zeros((2, 4, 128, 128), f32)
    for qi, w in enumerate((wq, wk)):
        for ch in range(4):
            for i in range(2):
                wqk_dr[qi, ch, :, i * 128 + 0 : i * 128 + E] = w[2 * ch][
                    i * 128 : (i + 1) * 128, :
                ]
                wqk_dr[qi, ch, :, i * 128 + EP : i * 128 + EP + E] = w[2 * ch + 1][
                    i * 128 : (i + 1) * 128, :
                ]
            wqk_p[qi, ch, :, 0:E] = w[2 * ch][256:384, :]
            wqk_p[qi, ch, :, EP : EP + E] = w[2 * ch + 1][256:384, :]

    # wv DR pair + plain: cols h*64+e
    wv_dr = np.zeros((128, 2 * DP), f32)
    wv_p = np.zeros((128, DP), f32)
    for h in range(H):
        for i in range(2):
            wv_dr[:, i * DP + h * EP : i * DP + h * EP + E] = wvv[h][
                i * 128 : (i + 1) * 128, :
            ]
        wv_p[:, h * EP : h * EP + E] = wvv[h][256:384, :]

    # wo DR: contraction = at chunk pairs; chunk a=0..3 rows: heads (2a, 2a+1)
    # 64-padded (head 2a at rows 0:48, head 2a+1 at 64:112)
    wo_dr = np.zeros((2, 128, 2 * D), f32)
    for a in range(4):
        j, i = a // 2, a % 2
        lo, hi = 2 * a * E, (2 * a + 1) * E
        wo_dr[j, 0:E, i * D : (i + 1) * D] = wo[lo : lo + E, :]
        wo_dr[j, EP : EP + E, i * D : (i + 1) * D] = wo[hi : hi + E, :]

    w1_c = np.zeros((KC, FC, 128, 128), f32)
    for k in range(KC):
        for f in range(FC):
            w1_c[k, f] = w1[k * 128 : (k + 1) * 128, f * 128 : (f + 1) * 128]

    w2_dr = np.zeros((6, 128, 2 * D), f32)
    for j in range(6):
        for i in range(2):
            w2_dr[j, :, i * D : (i + 1) * D] = w2[(2 * j + i) * 128 : (2 * j + i + 1) * 128, :]

    bqk = np.zeros((128, 8), f32)
    for qi, b in enumerate((bq, bk)):
        for ch in range(4):
            bqk[0:E, qi * 4 + ch] = b[2 * ch] * WS
            bqk[EP : EP + E, qi * 4 + ch] = b[2 * ch + 1] * WS
    bv_b = np.zeros((DP,), f32)
    for h in range(H):
        bv_b[h * EP : h * EP + E] = bv[h] * WS

    b1c = np.zeros((128, FC), f32)
    for f in range(FC):
        b1c[:, f] = b1[f * 128 : (f + 1) * 128]  # applied at 1/WS scale post-mm

    maskb = np.zeros((128, 4 * T), f32)
    s_idx = np.arange(T)
    for blk in range(4):
        for pp in range(128):
            maskb[pp, blk * T : (blk + 1) * T] = np.where(s_idx > (pp % T), MBIAS, 0.0)

    quake = np.zeros((128, 8), np.int32)
    quake[:, 0:4] = QUAKE_MAGIC
    quake[:, 4:8] = 1

    bcast = lambda v, w: np.broadcast_to(v[None, :], (128, w)).copy()

    flags = {
        "g1be1": bool(np.any(g1 != 1) or np.any(be1 != 0)),
        "g2be2": bool(np.any(g2 != 1) or np.any(be2 != 0)),
        "bqk": bool(np.any(bq) or np.any(bk)),
        "bv": bool(np.any(bv)),
        "bo": bool(np.any(bo)),
        "b1": bool(np.any(b1)),
        "b2": bool(np.any(b2)),
    }
    bf16 = ml_dtypes.bfloat16
    common = dict(
        ident1=np.eye(128, dtype=f32).astype(bf16),
        id4096=(np.eye(128, dtype=f32) * 4096.0).astype(bf16),
        id64=(np.eye(128, dtype=f32) * WS).astype(bf16),
        maskb=maskb.astype(bf16),
        wqk_dr=q8(wqk_dr),
        wqk_p=q8(wqk_p),
        wv_dr=q8(wv_dr),
        wv_p=q8(wv_p),
        wo_dr=q8(wo_dr),
        w1=w1_c.astype(bf16),
        w2_dr=q8(w2_dr),
        bqk=bqk,
        bv_b=bcast(bv_b, DP),
        b1c=b1c,
        g1_b=bcast(g1, D),
        be1_b=bcast(be1, D),
        g2_b=bcast(g2, D),
        be2_b=bcast(be2, D),
        bo_b=bcast(bo, D),
        b2_b=bcast(b2, D),
        quake=quake,
    )
    return common, flags


CONST_INFO = dict(
    ident1=((128, 128), BF16),
    id4096=((128, 128), BF16),
    id64=((128, 128), BF16),
    maskb=((128, 4 * T), BF16),
    wqk_dr=((2, 4, 128, 256), FP8),
    wqk_p=((2, 4, 128, 128), FP8),
    wv_dr=((128, 2 * DP), FP8),
    wv_p=((128, DP), FP8),
    wo_dr=((2, 128, 2 * D), FP8),
    w1=((KC, FC, 128, 128), BF16),
    w2_dr=((6, 128, 2 * D), FP8),
    bqk=((128, 8), F32),
    bv_b=((128, DP), F32),
    b1c=((128, FC), F32),
    g1_b=((128, D), F32),
    be1_b=((128, D), F32),
    g2_b=((128, D), F32),
    be2_b=((128, D), F32),
    bo_b=((128, D), F32),
    b2_b=((128, D), F32),
    quake=((128, 8), I32),
)


def build_program(b_core, flags):
    from concourse import bacc

    nc = bacc.Bacc("TRN2", target_bir_lowering=False, debug=False)
    aps = {}
    for name, (sh, dt) in CONST_INFO.items():
        aps[name] = nc.dram_tensor(name, list(sh), dt, kind="ExternalInput").ap()
    aps["x"] = nc.dram_tensor("x", [b_core, T, D], BF16, kind="ExternalInput").ap()
    aps["out"] = nc.dram_tensor("out", [b_core, T, D], F32, kind="ExternalOutput").ap()
    aps["flags"] = flags
    with tile.TileContext(nc) as tc:
        build_body(tc, aps, b_core)
    nc.compile()
    return nc


LAST_EXEC_NS = None


def kernel(**inputs):
    global LAST_EXEC_NS
    import ml_dtypes

    from concourse.bass_utils import run_bass_kernel_spmd

    x = np.ascontiguousarray(np.asarray(inputs["x"], np.float32)).astype(
        ml_dtypes.bfloat16
    )
    common, flags = prep_inputs(inputs, B_CORE)
    nc = build_program(B_CORE, flags)
    in_maps = []
    for c in range(N_CORES):
        m = dict(common)
        m["x"] = np.ascontiguousarray(x[c * B_CORE : (c + 1) * B_CORE])
        in_maps.append(m)
    res = run_bass_kernel_spmd(nc, in_maps, core_ids=list(range(N_CORES)))
    LAST_EXEC_NS = res.exec_time_ns
    out = np.concatenate([r["out"] for r in res.results], axis=0)
    return out.astype(np.float32)


# revision 7
# speedup vs baseline: 1.1314x; 1.1314x over previous
"""Trainium2 Bass kernel v9: dense transformer block (pre-LN, causal MHA+FFN).

Math per batch elem b (T=64, D=384): h=LN(x); per-head QKV; causal softmax;
attn@wo; h+=attn; h2=LN(h); out = h2 + relu(h2@w1)@w2.

Sharding: pure data parallel over batch (2048 -> 256/core, 8 cores SPMD).

v9 (from the v8 f32r baseline, HW 2.31ms -> ~1.6ms steady):
  - QKV/V/Wo/FFN2 GEMMs in fp8e4m3 with DoubleRow perf mode: contraction
    pairs packed 2/PE-cell (K=256/pass).  D=384 contractions run as one DR
    pass (256) + one plain-fp8 pass (128); DP=512 and F=1536 contractions
    are clean multiples of 256.  Weights pre-scaled x64 into fp8 range;
    descales folded into existing PSUM->SBUF copies / act scales.  FFN1
    stays bf16 -- it was the largest fp8 error contributor (rel err
    1.55e-2 -> 1.35e-2 vs the 2e-2 gate) and bf16 costs only ~2us/tile.
  - attention interior (scores, probs transposes, attnV) in bf16
    (1 cyc/row vs fp32's 4) with the same 64x64 PE-quadrant layout.
  - causal mask folded into the scores PSUM by an identity-matmul writing
    -1e30 biases (start of each accumulation group) -- no DVE mask multiply.
  - residuals folded into the Wo / FFN2 PSUM groups as scaled-identity
    matmuls (id*4096 @ h, id*64 @ h2) -- no DVE residual adds.
  - all token-major activation tiles bf16: DVE TensorScalar/Copy run in
    2x/4x perf modes; LN rstd via Quake+Newton on DVE (no Act table swaps;
    Act runs only Exp/Copy/Relu).
  - rotated software pipeline (emission order per iteration): pass2+Wo(it),
    LN1(it+1), transposes+QKV+V(it+1), LN2(it), h2T(it), scores+softmax
    (it+1), FFN(it).  Next-tile PE work fills the LN/softmax windows; buffer
    depths (p_h=10, p_qk/p_v=2, p_sm=8) keep the in-order engine streams
    from head-of-line blocking across tiles.
  - constant-weight DMAs round-robin over the SP/Act/GpSimd DGE queues so
    the ~60-load prologue overlaps (~15-20us one-time).
  - PSUM-drain engine placement tuned by isolated sim A/Bs (greedy walk,
    each step HW-directionally confirmed): V drains on Act (DVE is the
    pole in the LN1/attention window); relu drains all on Act; FFN2-out
    and Wo-out (hr) drains on DVE (Act is the pole in the FFN window).
    HW 1.502 ms, rel err 1.345e-02.
"""

import sys

sys.path.insert(0, "/opt/trn_rl_repo")

import numpy as np

import concourse.bass as bass
import concourse.tile as tile
from concourse import mybir

# ---- problem constants (hardcoded per contract) ----
B_TOTAL = 2048
T = 64
D = 384
H = 8
E = 48
EP = 64
F = 4 * D
N_CORES = 8
B_CORE = B_TOTAL // N_CORES
LN_EPS = 1e-5
INV_SQRT_E = float(E) ** -0.5

NB = 8
NT = NB * T  # 512
KC = D // 128  # 3
FC = F // 128  # 12
TC = NT // 128  # 4
DP = H * EP  # 512
WS = 64.0  # fp8 weight pre-scale
MBIAS = -1e30  # causal mask bias (pre exp-scale)

F32 = mybir.dt.float32
I32 = mybir.dt.int32
BF16 = mybir.dt.bfloat16
FP8 = mybir.dt.float8e4

QUAKE_MAGIC = 0x5F3759DF
DRM = mybir.MatmulPerfMode.DoubleRow


def build_body(tc, aps, b_core):
    from contextlib import ExitStack

    ctx = ExitStack()
    nc = tc.nc
    n_tiles = b_core * T // NT

    x_dr = aps["x"].rearrange("b t d -> (b t) d")
    out_dr = aps["out"].rearrange("b t d -> (b t) d")

    AF = mybir.ActivationFunctionType
    OP = mybir.AluOpType
    flags = aps["flags"]

    singles = ctx.enter_context(tc.tile_pool(name="singles", bufs=1))

    _ldq = [nc.sync, nc.scalar, nc.gpsimd]
    _ldn = [0]

    def load_const(name, shape, src_ap, dt):
        t_ = singles.tile(list(shape), dt, name=f"sb_{name}")
        eng = _ldq[_ldn[0] % len(_ldq)]
        _ldn[0] += 1
        eng.dma_start(out=t_, in_=src_ap)
        return t_

    ident1 = load_const("ident1", [128, 128], aps["ident1"], BF16)
    id4096 = load_const("id4096", [128, 128], aps["id4096"], BF16)
    id64 = load_const("id64", [128, 128], aps["id64"], BF16)
    maskb = load_const("maskb", [128, 4 * T], aps["maskb"], BF16)
    quake = load_const("quake", [128, 8], aps["quake"], I32)

    wqk_dr = {
        (qi, ch): load_const(f"wqkd{qi}{ch}", [128, 256], aps["wqk_dr"][qi, ch], FP8)
        for qi in range(2)
        for ch in range(4)
    }
    wqk_p = {
        (qi, ch): load_const(f"wqkp{qi}{ch}", [128, 128], aps["wqk_p"][qi, ch], FP8)
        for qi in range(2)
        for ch in range(4)
    }
    wv_dr = load_const("wvd", [128, 2 * DP], aps["wv_dr"], FP8)
    wv_p = load_const("wvp", [128, DP], aps["wv_p"], FP8)
    w1 = {
        (k, f): load_const(f"w1_{k}_{f}", [128, 128], aps["w1"][k, f], BF16)
        for k in range(KC)
        for f in range(FC)
    }
    w2_dr = {
        j: load_const(f"w2d{j}", [128, 2 * D], aps["w2_dr"][j], FP8) for j in range(6)
    }
    wo_dr = {
        j: load_const(f"wod{j}", [128, 2 * D], aps["wo_dr"][j], FP8) for j in range(2)
    }

    bqk = load_const("bqk", [128, 8], aps["bqk"], F32) if flags["bqk"] else None
    bv_b = load_const("bv_b", [128, DP], aps["bv_b"], F32) if flags["bv"] else None
    b1c = load_const("b1c", [128, FC], aps["b1c"], F32) if flags["b1"] else None
    g1_b = load_const("g1_b", [128, D], aps["g1_b"], F32) if flags["g1be1"] else None
    be1_b = load_const("be1_b", [128, D], aps["be1_b"], F32) if flags["g1be1"] else None
    g2_b = load_const("g2_b", [128, D], aps["g2_b"], F32) if flags["g2be2"] else None
    be2_b = load_const("be2_b", [128, D], aps["be2_b"], F32) if flags["g2be2"] else None
    bo_b = load_const("bo_b", [128, D], aps["bo_b"], F32) if flags["bo"] else None
    b2_b = load_const("b2_b", [128, D], aps["b2_b"], F32) if flags["b2"] else None

    pool = lambda nm, n, **kw: ctx.enter_context(tc.tile_pool(name=nm, bufs=n, **kw))
    ps = pool("ps", 4, space="PSUM")
    ps_at = pool("ps_at", 1, space="PSUM")
    p_x = pool("p_x", 4)
    p_h = pool("p_h", 10)  # 4/tile; stage_a(it+1) overlaps Wo(it) readers
    p_hT = pool("p_hT", 1)  # tags hT01/hT2/h2T01/h2T2
    p_qk = pool("p_qk", 2)  # 8 tags; next tile's QKV overlaps scores(it)
    p_v = pool("p_v", 2)  # 8 tags; next tile's V overlaps attnV(it)
    p_sm = pool("p_sm", 8)
    p_pt = pool("p_pt", 3)
    p_at = pool("p_at", 1)  # 2 tags
    p_hr = pool("p_hr", 5)
    p_h2 = pool("p_h2", 6)
    p_rel = pool("p_rel", 1)  # 6 tags
    p_out = pool("p_out", 2)
    p_st = pool("p_st", 4)
    def rsqrt_dve(ve_view, n, tag):
        """rstd[128, n] = 1/sqrt(ve + eps): Quake bitcast + 2 Newton on DVE."""
        ve = p_st.tile([128, n], F32, tag=f"q0{tag}", name=f"q0{tag}")
        nc.vector.tensor_scalar_add(out=ve, in0=ve_view, scalar1=LN_EPS)
        t1 = p_st.tile([128, n], I32, tag=f"q1{tag}", name=f"q1{tag}")
        nc.vector.tensor_tensor(
            out=t1, in0=ve.bitcast(I32), in1=quake[:, 4 : 4 + n], op=OP.logical_shift_right
        )
        y0 = p_st.tile([128, n], F32, tag=f"q2{tag}", name=f"q2{tag}")
        nc.vector.tensor_tensor(
            out=y0.bitcast(I32), in0=quake[:, 0:n], in1=t1, op=OP.subtract
        )
        y = y0
        for ni in range(2):
            a = p_st.tile([128, n], F32, tag=f"q3{tag}{ni}", name=f"q3{tag}{ni}")
            nc.vector.tensor_mul(out=a, in0=y, in1=y)
            b = p_st.tile([128, n], F32, tag=f"q4{tag}{ni}", name=f"q4{tag}{ni}")
            nc.vector.scalar_tensor_tensor(
                out=b, in0=a, scalar=-0.5, in1=ve, op0=OP.mult, op1=OP.mult
            )
            c = p_st.tile([128, n], F32, tag=f"q5{tag}{ni}", name=f"q5{tag}{ni}")
            nc.vector.tensor_scalar_add(out=c, in0=b, scalar1=1.5)
            yn = p_st.tile([128, n], F32, tag=f"q6{tag}{ni}", name=f"q6{tag}{ni}")
            nc.vector.tensor_mul(out=yn, in0=y, in1=c)
            y = yn
        return y

    def layernorm_group(x_ts, g_b, be_b, gb_nontriv, pool_, tag):
        """LN over TC token chunks -> bf16 outputs; batched rstd."""
        mv = p_st.tile([128, 2 * TC], F32, tag=f"mv{tag}", name=f"mv_{tag}")
        for c in range(TC):
            st = p_st.tile([128, 6], F32, tag="st", name=f"st_{tag}")
            nc.vector.bn_stats(out=st, in_=x_ts[c])
            nc.vector.bn_aggr(out=mv[:, 2 * c : 2 * c + 2], in_=st)
        mv3 = mv.rearrange("p (c two) -> p c two", two=2)
        rstd = rsqrt_dve(mv3[:, :, 1], TC, tag)
        nmr = p_st.tile([128, TC], F32, tag=f"nm{tag}", name=f"nm_{tag}")
        nc.vector.scalar_tensor_tensor(
            out=nmr, in0=mv3[:, :, 0], scalar=-1.0, in1=rstd, op0=OP.mult, op1=OP.mult
        )
        h_ts = []
        for c in range(TC):
            h_t = pool_.tile([128, D], BF16, tag=tag, name=f"h_{tag}")
            nc.vector.tensor_scalar(
                out=h_t,
                in0=x_ts[c],
                scalar1=rstd[:, c : c + 1],
                scalar2=nmr[:, c : c + 1],
                op0=OP.mult,
                op1=OP.add,
            )
            if gb_nontriv:
                nc.vector.tensor_mul(out=h_t, in0=h_t, in1=g_b)
                nc.vector.tensor_add(out=h_t, in0=h_t, in1=be_b)
            h_ts.append(h_t)
        return h_ts

    def transpose_feat(h_ts, tag, dt=FP8):
        """token-major bf16 [128, D] x TC -> feature-major: pair tile
        [128, 2*NT] (chunks 0,1) + single tile [128, NT] (chunk 2).
        Each PSUM->SBUF copy is split Act/DVE so the copy pole is short."""
        t01 = p_hT.tile([128, 2 * NT], dt, tag=f"{tag}01", name=f"{tag}01")
        t2 = p_hT.tile([128, NT], dt, tag=f"{tag}2", name=f"{tag}2")
        for k in range(KC):
            pt_ = ps.tile([128, NT], BF16, tag="ps", name=f"tp_{tag}")
            for c in range(TC):
                nc.tensor.transpose(
                    out=pt_[:, c * 128 : (c + 1) * 128],
                    in_=h_ts[c][:, k * 128 : (k + 1) * 128],
                    identity=ident1,
                )
            dst = t2 if k == 2 else t01[:, k * NT : (k + 1) * NT]
            if dt == FP8:
                nc.scalar.copy(out=dst, in_=pt_)
            else:
                nc.vector.tensor_copy(out=dst, in_=pt_)
        return t01.rearrange("p (two n) -> p two n", two=2), t2

    x_tiles = {}

    def load_x(i):
        if i >= n_tiles:
            return
        t = p_x.tile([128, TC * D], BF16, tag="x", name="x")
        nc.sync.dma_start(
            out=t.rearrange("p (c d) -> p c d", c=TC),
            in_=x_dr[i * NT : (i + 1) * NT, :].rearrange("(c p) d -> p c d", p=128),
        )
        x_tiles[i] = t

    wv_dr3 = wv_dr.rearrange("p (two n) -> p two n", two=2)

    def stage_a_ln(it):
        """LN1 for tile `it` (DVE only)."""
        x_big = x_tiles.pop(it)
        x_ts = [x_big[:, c * D : (c + 1) * D] for c in range(TC)]
        return layernorm_group(x_ts, g1_b, be1_b, flags["g1be1"], p_h, "h")

    def stage_a_qkv(h_ts):
        """transposes + QKV + V from LN1 output."""
        hT01, hT2 = transpose_feat(h_ts, "hT")

        # Q,K: feature-major, 64-padded heads; chunk ch = heads (2ch, 2ch+1)
        qk_sb = []
        for qi in range(2):
            row = []
            for ch in range(4):
                pm = ps.tile([128, NT], F32, tag="ps", name="qk_ps")
                nc.tensor.matmul(
                    out=pm,
                    lhsT=wqk_dr[(qi, ch)].rearrange("p (two m) -> p two m", two=2),
                    rhs=hT01,
                    start=True,
                    stop=False,
                    perf_mode=DRM,
                )
                nc.tensor.matmul(
                    out=pm, lhsT=wqk_p[(qi, ch)], rhs=hT2, start=False, stop=True
                )
                sb = p_qk.tile([128, NT], BF16, tag=f"qk{qi}{ch}", name=f"qk{qi}{ch}")
                if flags["bqk"]:
                    nc.scalar.activation(
                        out=sb,
                        in_=pm,
                        func=AF.Identity,
                        bias=bqk[:, qi * 4 + ch : qi * 4 + ch + 1],
                        scale=1.0,
                    )
                else:
                    nc.scalar.copy(out=sb, in_=pm)
                row.append(sb)
            qk_sb.append(row)

        # V: token-major [128 tok, DP]; values carry xWS
        v_sb, vs_sb = [], []
        for c in range(TC):
            pm = ps.tile([128, DP], F32, tag="ps", name="v_ps")
            nc.tensor.matmul(
                out=pm,
                lhsT=hT01[:, :, c * 128 : (c + 1) * 128],
                rhs=wv_dr3,
                start=True,
                stop=False,
                perf_mode=DRM,
            )
            nc.tensor.matmul(
                out=pm,
                lhsT=hT2[:, c * 128 : (c + 1) * 128],
                rhs=wv_p,
                start=False,
                stop=True,
            )
            sb = p_v.tile([128, DP], BF16, tag=f"v{c}", name=f"v{c}")
            nc.vector.tensor_copy(out=sb, in_=pm)
            if flags["bv"]:
                nc.vector.tensor_add(out=sb, in0=sb, in1=bv_b)
            sw = p_v.tile([128, DP], BF16, tag=f"vs{c}", name=f"vs{c}")
            # half-swapped copy; bf16 SBUF->SBUF runs 4x on DVE
            nc.vector.tensor_copy(out=sw[0:64, :], in_=sb[64:128, :])
            nc.vector.tensor_copy(out=sw[64:128, :], in_=sb[0:64, :])
            v_sb.append(sb)
            vs_sb.append(sw)
        return h_ts, qk_sb, v_sb, vs_sb

    load_x(0)
    load_x(1)
    load_x(2)

    exp_scale = INV_SQRT_E / (WS * WS)

    def pass1(it, sa_):
        """scores + softmax for tile it (probs stay x1, v carries xWS)."""
        _, qk_sb, _, _ = sa_
        exs = []
        for p in range(NB // 2):
            sc_par = [
                ps.tile([128, 4 * T], F32, tag="ps", name=f"sc_ps{par}")
                for par in range(2)
            ]
            for par in range(2):
                # causal mask bias written by PE: I.T @ maskb
                nc.tensor.matmul(
                    out=sc_par[par], lhsT=ident1, rhs=maskb, start=True, stop=False
                )
            for half in range(2):
                bb = 2 * p + half
                for h in range(H):
                    ch, off = h // 2, EP * (h % 2)
                    nc.tensor.matmul(
                        out=sc_par[h % 2][
                            64 * half : 64 * half + 64, (h // 2) * T : (h // 2 + 1) * T
                        ],
                        lhsT=qk_sb[0][ch][off : off + E, bb * T : (bb + 1) * T],
                        rhs=qk_sb[1][ch][off : off + E, bb * T : (bb + 1) * T],
                        start=False,
                        stop=(half == 1 and h >= H - 2),
                    )
            ex = p_sm.tile([128, 8 * T], BF16, tag="ex", name="ex")
            for par in range(2):
                nc.scalar.activation(
                    out=ex[:, par * 4 * T : (par + 1) * 4 * T],
                    in_=sc_par[par],
                    func=AF.Exp,
                    bias=0.0,
                    scale=exp_scale,
                )
            rs = p_st.tile([128, H], F32, tag="rsum", name="rsum")
            nc.vector.reduce_sum(
                out=rs,
                in_=ex.rearrange("p (h s) -> p h s", h=H),
                axis=mybir.AxisListType.X,
            )
            rr = p_st.tile([128, H], F32, tag="rrec", name="rrec")
            nc.vector.reciprocal(out=rr, in_=rs)
            for h in range(H):
                nc.vector.tensor_scalar_mul(
                    out=ex[:, h * T : (h + 1) * T],
                    in0=ex[:, h * T : (h + 1) * T],
                    scalar1=rr[:, h : h + 1],
                )
            exs.append(ex)
        return exs

    def pass2_wo(it, sa_, exs):
        """probs transposes + attnV, with per-token-chunk at copies and Wo
        groups interleaved so the PE stream never waits on a full drain."""
        h_ts, _, v_sb, vs_sb = sa_
        at_ps = {
            (g, cg): ps_at.tile([128, NT], F32, tag=f"at{g}{cg}", name=f"at{g}{cg}")
            for g in range(2)
            for cg in range(2)
        }
        at01 = p_at.tile([128, 2 * NT], FP8, tag="at01", name="at01")
        at23 = p_at.tile([128, 2 * NT], FP8, tag="at23", name="at23")
        at01_3 = at01.rearrange("p (two n) -> p two n", two=2)
        at23_3 = at23.rearrange("p (two n) -> p two n", two=2)
        hr_ts = []

        def wo_group(c):
            pm = ps.tile([128, D], F32, tag="ps", name="wo_ps")
            nc.tensor.matmul(
                out=pm,
                lhsT=at01_3[:, :, c * 128 : (c + 1) * 128],
                rhs=wo_dr[0].rearrange("p (two n) -> p two n", two=2),
                start=True,
                stop=False,
                perf_mode=DRM,
            )
            nc.tensor.matmul(
                out=pm,
                lhsT=at23_3[:, :, c * 128 : (c + 1) * 128],
                rhs=wo_dr[1].rearrange("p (two n) -> p two n", two=2),
                start=False,
                stop=False,
                perf_mode=DRM,
            )
            nc.tensor.matmul(out=pm, lhsT=id4096, rhs=h_ts[c], start=False, stop=True)
            hr = p_hr.tile([128, D], BF16, tag="hr", name="hr")
            nc.vector.tensor_scalar_mul(out=hr, in0=pm, scalar1=1.0 / 4096.0)
            if flags["bo"]:
                nc.vector.tensor_add(out=hr, in0=hr, in1=bo_b)
            hr_ts.append(hr)

        for p in range(NB // 2):
            ex = exs[p]
            ptp = ps.tile([128, 8 * T], BF16, tag="ps", name="pt_ps")
            for j2 in range(4):
                nc.tensor.transpose(
                    out=ptp[:, j2 * 128 : (j2 + 1) * 128],
                    in_=ex[:, j2 * 128 : (j2 + 1) * 128],
                    identity=ident1,
                )
            ptsb = p_pt.tile([128, 8 * T], BF16, tag="pt", name="pt")
            if p % 2 == 0:
                nc.scalar.copy(out=ptsb, in_=ptp)
            else:
                nc.vector.tensor_copy(out=ptsb, in_=ptp)
            for half in range(2):
                bb = 2 * p + half
                c, hb = bb // 2, 64 * (bb % 2)
                for h in range(H):
                    ch = h // 2
                    j = (h % 2) * 4 + ch
                    pbase = 64 * (j % 2)
                    vt = v_sb[c] if (j % 2) == (bb % 2) else vs_sb[c]
                    nc.tensor.matmul(
                        out=at_ps[(ch % 2, ch // 2)][
                            EP * (h % 2) : EP * (h % 2) + EP,
                            bb * T : (bb + 1) * T,
                        ],
                        lhsT=vt[pbase : pbase + 64, h * EP : (h + 1) * EP],
                        rhs=ptsb[
                            pbase : pbase + 64,
                            (j // 2) * 128 + hb : (j // 2) * 128 + hb + 64,
                        ],
                        start=True,
                        stop=True,
                    )
        nc.scalar.copy(out=at01[:, 0:NT], in_=at_ps[(0, 0)])
        nc.vector.tensor_copy(out=at01[:, NT : 2 * NT], in_=at_ps[(1, 0)])
        nc.scalar.copy(out=at23[:, 0:NT], in_=at_ps[(0, 1)])
        nc.vector.tensor_copy(out=at23[:, NT : 2 * NT], in_=at_ps[(1, 1)])
        for c in range(TC):
            wo_group(c)
        return hr_ts

    def ffn_store(it, h2_ts, h2T):
        """FFN1 (bf16) + relu -> fp8 pairs; FFN2 (DR) + residual; store."""
        row0 = it * NT
        rel3 = []
        for j in range(6):
            rp = p_rel.tile([128, 2 * NT], FP8, tag=f"rel{j}", name=f"rel{j}")
            rel3.append(rp)
        for f in range(FC):
            pm = ps.tile([128, NT], F32, tag="ps", name="f1_ps")
            for k in range(KC):
                nc.tensor.matmul(
                    out=pm,
                    lhsT=w1[(k, f)],
                    rhs=h2T[k],
                    start=(k == 0),
                    stop=(k == KC - 1),
                )
            dst = rel3[f // 2][:, (f % 2) * NT : (f % 2 + 1) * NT]
            if flags["b1"]:
                nc.scalar.activation(
                    out=dst, in_=pm, func=AF.Relu, bias=b1c[:, f : f + 1], scale=1.0
                )
            else:
                nc.scalar.activation(out=dst, in_=pm, func=AF.Relu, bias=0.0, scale=1.0)

        # ---- FFN2 + residual (id*64 @ h2) + wide store ----
        o_big = p_out.tile([128, TC * D], F32, tag="o", name="o")
        for c in range(TC):
            pm = ps.tile([128, D], F32, tag="ps", name="f2_ps")
            for j in range(6):
                nc.tensor.matmul(
                    out=pm,
                    lhsT=rel3[j].rearrange("p (two n) -> p two n", two=2)[
                        :, :, c * 128 : (c + 1) * 128
                    ],
                    rhs=w2_dr[j].rearrange("p (two n) -> p two n", two=2),
                    start=(j == 0),
                    stop=False,
                    perf_mode=DRM,
                )
            nc.tensor.matmul(out=pm, lhsT=id64, rhs=h2_ts[c], start=False, stop=True)
            o_t = o_big[:, c * D : (c + 1) * D]
            nc.vector.tensor_scalar_mul(out=o_t, in0=pm, scalar1=1.0 / WS)
            if flags["b2"]:
                nc.vector.tensor_add(out=o_t, in0=o_t, in1=b2_b)
        nc.sync.dma_start(
            out=out_dr[row0 : row0 + NT, :].rearrange("(c p) d -> p c d", p=128),
            in_=o_big.rearrange("p (c d) -> p c d", c=TC),
        )

    # ---- rotated pipeline.  Emission order per iteration:
    #   pass2_wo(it): probsT/attnV with per-chunk at drains + Wo interleaved
    #   LN1(it+1) on DVE (runs under pass2/Wo PE work)
    #   stage_a_qkv(it+1): transposes+QKV+V PE stream
    #   LN2(it) on DVE (runs under QKV/scores PE work)
    #   pass1(it+1): scores fill the LN2 window; softmax overlaps FFN(it)
    #   h2T transposes + FFN(it)
    sa = {0: stage_a_qkv(stage_a_ln(0))}
    exs = {0: pass1(0, sa[0])}
    for it in range(n_tiles):
        load_x(it + 3)
        hr_ts = pass2_wo(it, sa[it], exs.pop(it))
        h_next = stage_a_ln(it + 1) if it + 1 < n_tiles else None
        if h_next is not None:
            sa[it + 1] = stage_a_qkv(h_next)
        h2_ts = layernorm_group(hr_ts, g2_b, be2_b, flags["g2be2"], p_h2, "h2")
        h2T01, h2T2 = transpose_feat(h2_ts, "h2T", dt=BF16)
        if it + 1 < n_tiles:
            exs[it + 1] = pass1(it + 1, sa[it + 1])
        ffn_store(it, h2_ts, [h2T01[:, 0, :], h2T01[:, 1, :], h2T2])
        sa.pop(it)

    ctx.close()


def prep_inputs(inputs, b_core):
    import ml_dtypes

    f32 = np.float32
    fp8 = ml_dtypes.float8_e4m3fn
    wq, wk, wvv = (np.asarray(inputs[k], f32) for k in ("wq", "wk", "wv"))
    bq, bk, bv = (np.asarray(inputs[k], f32) for k in ("bq", "bk", "bv"))
    wo, bo = np.asarray(inputs["wo"], f32), np.asarray(inputs["bo"], f32)
    w1, b1 = np.asarray(inputs["w1"], f32), np.asarray(inputs["b1"], f32)
    w2, b2 = np.asarray(inputs["w2"], f32), np.asarray(inputs["b2"], f32)
    g1, be1 = np.asarray(inputs["g1"], f32), np.asarray(inputs["be1"], f32)
    g2, be2 = np.asarray(inputs["g2"], f32), np.asarray(inputs["be2"], f32)

    def q8(a):
        return (a * WS).astype(fp8)

    # wqk DR pair (contraction chunks 0,1) + plain (chunk 2); head-pair cols
    wqk_dr = np.zeros((2, 4, 128, 256), f32)
    wqk_p = np.zeros((2, 4, 128, 128), f32)
    for qi, w in enumerate((wq, wk)):
        for ch in range(4):
            for i in range(2):
                wqk_dr[qi, ch, :, i * 128 + 0 : i * 128 + E] = w[2 * ch][
                    i * 128 : (i + 1) * 128, :
                ]
                wqk_dr[qi, ch, :, i * 128 + EP : i * 128 + EP + E] = w[2 * ch + 1][
                    i * 128 : (i + 1) * 128, :
                ]
            wqk_p[qi, ch, :, 0:E] = w[2 * ch][256:384, :]
            wqk_p[qi, ch, :, EP : EP + E] = w[2 * ch + 1][256:384, :]

    # wv DR pair + plain: cols h*64+e
    wv_dr = np.zeros((128, 2 * DP), f32)
    wv_p = np.zeros((128, DP), f32)
    for h in range(H):
        for i in range(2):
            wv_dr[:, i * DP + h * EP : i * DP + h * EP + E] = wvv[h][
                i * 128 : (i + 1) * 128, :
            ]
        wv_p[:, h * EP : h * EP + E] = wvv[h][256:384, :]

    # wo DR: contraction = at chunk pairs; chunk a=0..3 rows: heads (2a, 2a+1)
    # 64-padded (head 2a at rows 0:48, head 2a+1 at 64:112)
    wo_dr = np.zeros((2, 128, 2 * D), f32)
    for a in range(4):
        j, i = a // 2, a % 2
        lo, hi = 2 * a * E, (2 * a + 1) * E
        wo_dr[j, 0:E, i * D : (i + 1) * D] = wo[lo : lo + E, :]
        wo_dr[j, EP : EP + E, i * D : (i + 1) * D] = wo[hi : hi + E, :]

    w1_c = np.zeros((KC, FC, 128, 128), f32)
    for k in range(KC):
        for f in range(FC):
            w1_c[k, f] = w1[k * 128 : (k + 1) * 128, f * 128 : (f + 1) * 128]

    w2_dr = np.zeros((6, 128, 2 * D), f32)
    for j in range(6):
        for i in range(2):
            w2_dr[j, :, i * D : (i + 1) * D] = w2[(2 * j + i) * 128 : (2 * j + i + 1) * 128, :]

    bqk = np.zeros((128, 8), f32)
    for qi, b in enumerate((bq, bk)):
        for ch in range(4):
            bqk[0:E, qi * 4 + ch] = b[2 * ch] * WS
            bqk[EP : EP + E, qi * 4 + ch] = b[2 * ch + 1] * WS
    bv_b = np.zeros((DP,), f32)
    for h in range(H):
        bv_b[h * EP : h * EP + E] = bv[h] * WS

    b1c = np.zeros((128, FC), f32)
    for f in range(FC):
        b1c[:, f] = b1[f * 128 : (f + 1) * 128]  # applied at 1/WS scale post-mm

    maskb = np.zeros((128, 4 * T), f32)
    s_idx = np.arange(T)
    for blk in range(4):
        for pp in range(128):
            maskb[pp, blk * T : (blk + 1) * T] = np.where(s_idx > (pp % T), MBIAS, 0.0)

    quake = np.zeros((128, 8), np.int32)
    quake[:, 0:4] = QUAKE_MAGIC
    quake[:, 4:8] = 1

    bcast = lambda v, w: np.broadcast_to(v[None, :], (128, w)).copy()

    flags = {
        "g1be1": bool(np.any(g1 != 1) or np.any(be1 != 0)),
        "g2be2": bool(np.any(g2 != 1) or np.any(be2 != 0)),
        "bqk": bool(np.any(bq) or np.any(bk)),
        "bv": bool(np.any(bv)),
        "bo": bool(np.any(bo)),
        "b1": bool(np.any(b1)),
        "b2": bool(np.any(b2)),
    }
    bf16 = ml_dtypes.bfloat16
    common = dict(
        ident1=np.eye(128, dtype=f32).astype(bf16),
        id4096=(np.eye(128, dtype=f32) * 4096.0).astype(bf16),
        id64=(np.eye(128, dtype=f32) * WS).astype(bf16),
        maskb=maskb.astype(bf16),
        wqk_dr=q8(wqk_dr),
        wqk_p=q8(wqk_p),
        wv_dr=q8(wv_dr),
        wv_p=q8(wv_p),
        wo_dr=q8(wo_dr),
        w1=w1_c.astype(bf16),
        w2_dr=q8(w2_dr),
        bqk=bqk,
        bv_b=bcast(bv_b, DP),
        b1c=b1c,
        g1_b=bcast(g1, D),
        be1_b=bcast(be1, D),
        g2_b=bcast(g2, D),
        be2_b=bcast(be2, D),
        bo_b=bcast(bo, D),
        b2_b=bcast(b2, D),
        quake=quake,
    )
    return common, flags


CONST_INFO = dict(
    ident1=((128, 128), BF16),
    id4096=((128, 128), BF16),
    id64=((128, 128), BF16),
    maskb=((128, 4 * T), BF16),
    wqk_dr=((2, 4, 128, 256), FP8),
    wqk_p=((2, 4, 128, 128), FP8),
    wv_dr=((128, 2 * DP), FP8),
    wv_p=((128, DP), FP8),
    wo_dr=((2, 128, 2 * D), FP8),
    w1=((KC, FC, 128, 128), BF16),
    w2_dr=((6, 128, 2 * D), FP8),
    bqk=((128, 8), F32),
    bv_b=((128, DP), F32),
    b1c=((128, FC), F32),
    g1_b=((128, D), F32),
    be1_b=((128, D), F32),
    g2_b=((128, D), F32),
    be2_b=((128, D), F32),
    bo_b=((128, D), F32),
    b2_b=((128, D), F32),
    quake=((128, 8), I32),
)


def build_program(b_core, flags):
    from concourse import bacc

    nc = bacc.Bacc("TRN2", target_bir_lowering=False, debug=False)
    aps = {}
    for name, (sh, dt) in CONST_INFO.items():
        aps[name] = nc.dram_tensor(name, list(sh), dt, kind="ExternalInput").ap()
    aps["x"] = nc.dram_tensor("x", [b_core, T, D], BF16, kind="ExternalInput").ap()
    aps["out"] = nc.dram_tensor("out", [b_core, T, D], F32, kind="ExternalOutput").ap()
    aps["flags"] = flags
    with tile.TileContext(nc) as tc:
        build_body(tc, aps, b_core)
    nc.compile()
    return nc


LAST_EXEC_NS = None


def kernel(**inputs):
    global LAST_EXEC_NS
    import ml_dtypes

    from concourse.bass_utils import run_bass_kernel_spmd

    x = np.ascontiguousarray(np.asarray(inputs["x"], np.float32)).astype(
        ml_dtypes.bfloat16
    )
    common, flags = prep_inputs(inputs, B_CORE)
    nc = build_program(B_CORE, flags)
    in_maps = []
    for c in range(N_CORES):
        m = dict(common)
        m["x"] = np.ascontiguousarray(x[c * B_CORE : (c + 1) * B_CORE])
        in_maps.append(m)
    res = run_bass_kernel_spmd(nc, in_maps, core_ids=list(range(N_CORES)))
    LAST_EXEC_NS = res.exec_time_ns
    out = np.concatenate([r["out"] for r in res.results], axis=0)
    return out.astype(np.float32)
